# revision 1
# baseline (speedup 1.0000x reference)
"""Distributed Trainium2 (8 NeuronCores) kernel for the 3-node ConvGRU
message-passing network.

Strategy (memory-bound: the five big projection matrices dominate traffic):
  - All-bf16 datapath: projection weights are host-converted to bf16 and
    tensor-sharded across the 8 cores by output feature, stored partition-
    major [98, K, O/8] so each streaming DMA reads contiguous slabs.
    Halves HBM traffic and host->device bytes vs f32.
  - Activations, gather bounce, conv weights and the h state are bf16
    (PSUM accumulation stays f32); end-to-end error ~8.7e-3 (gate 2e-2).
  - Convs: 6 accumulating matmul passes per conv (3x K=112 pair-taps using
    an x+1-shifted copy of the input stacked on partitions 64:112, plus 3x
    K=48 single taps) instead of 9 passes of K=48.
  - Big matmuls: lhsT = transposed activations [98, B] per (s, ch) chunk
    (s-major, so the first half of each contraction only needs maxpool
    slab 0), rhs = streamed bf16 weight tiles; outputs transposed o-major,
    biased, and AllGathered ([O, B] bf16 bounce) twice per timestep:
    B(u)={td1(u),bu1(u)} -> cell1(u); A(u)={bu2(u),bu0(u+1),td0(u+1)} ->
    cell2(u), cell0(u+1).
  - Latency hiding: bu0(u+1) (x-only input) fills the gather-B window;
    cell0(u+1) is deferred and its conv chunks interleave with td1(u+1)'s
    matmul stream (generator zip); maxpool+transpose is pipelined by slab
    (y-pass/repack/PE-transpose of slab 1 overlaps the first 32 matmul
    chunks via big_matmul's mid= hook); the maxpool x-pass is emitted per
    y-half directly behind the GRU h update; all cell elementwise work is
    split into y-halves to shorten the serial chain into each conv.
  - Queue discipline: weight streams on SP/HWDGE, bounce writes colocated
    with the collectives on the gpsimd/SWDGE queue, reloads + shifted
    copies + PSUM evacuations on the ACT queue, maxpool/assembly on DVE.
  - fc1 is output-sharded (13 of 104 padded outputs per core, full
    contraction) with one tiny f32 partial AllGather at the end, so the
    fc1 matrix isn't replicated across cores.

Measured (8-core HW): ~250-330 us/step with collectives, ~240 us/step
without (f32 baseline: ~600-700 us/step, same estimator); single-core
TimelineSim 2.32 ms for the full t_end=10 run vs 4.86 ms for the f32
baseline (~2.1x compute). Per-projection PSUM accumulators are split
(td 3-bank / bu 1-bank tags) and cell0 gets 2 zip steps per td1 group.

Self-contained: hardcodes all shapes; host-side numpy does the sharding,
permutation, bf16 conversion and final unshard.
"""
import sys
import numpy as np
import ml_dtypes

for _p in ("/opt/trn_rl_repo", "/opt/pypackages",
           "/root/.axon_site", "/root/.axon_site/_ro/trn_rl_repo",
           "/root/.axon_site/_ro/pypackages"):
    if _p not in sys.path:
        sys.path.append(_p)

import concourse.bass as bass
import concourse.bacc as bacc
import concourse.mybir as mybir
import concourse.tile as tile
from concourse import bass_utils

F32 = mybir.dt.float32
F32R = mybir.dt.float32r
BF16 = mybir.dt.bfloat16
AF = mybir.ActivationFunctionType
GDT = BF16                   # dtype of the gather path (bounce + reload)
NPBF = ml_dtypes.bfloat16

NCORES = 8
B, T, C, H, W = 16, 8, 3, 14, 14
HID, IND, N = 32, 16, 3
CIN = IND + HID              # 48 conv input channels
YP = XP = 16                 # padded spatial
# conv valid output flat window (phys coords, (y*XP+x)*B): (1,1)..(14,14)
WSTART = (1 * XP + 1) * B
WLEN = ((14 * XP + 14) - (1 * XP + 1) + 1) * B    # 3552
FLAT = YP * XP * B           # 4096

KP = 98                      # partitions per feature chunk (7 y-rows x 14 x)
KH = 2 * HID                 # 64 chunks for hidden-sized contraction (6272)
KX = 2 * C                   # 6 chunks for x contraction (588)
O_TD = (IND + HID) * H * W   # 9408
O_BU = IND * H * W           # 3136
OTD8 = O_TD // NCORES        # 1176 = 6 channels
OBU8 = O_BU // NCORES        # 392  = 2 channels
NJ_TD = (OTD8 + 127) // 128  # 10 o-chunks
NJ_BU = (OBU8 + 127) // 128  # 4
GRP_TD = 8                   # weight K-chunks per DMA (td)
GRP_BU = 16
OFC = 13                     # fc1 output columns per core (8*13=104 >= 100)
KPAIR = 112                  # pair-tap conv K: 48 + 16 zero pad + 48 shifted

_CACHED = {}


# ---------------------------------------------------------------- graph ----
def build_graph(t_end=T + N - 1, debug_h=False, no_cc=False):
    nc = bacc.Bacc(None, target_bir_lowering=False, debug=False,
                   num_devices=NCORES)

    dp = nc.declare_dram_parameter
    # streamed weight shards, partition-major [98, K, O/8] bf16
    tw0 = dp("tw0", [KP, KH, OTD8], BF16, isOutput=False)
    tw1 = dp("tw1", [KP, KH, OTD8], BF16, isOutput=False)
    bw0 = dp("bw0", [KP, KX, OBU8], BF16, isOutput=False)
    bw1 = dp("bw1", [KP, KH, OBU8], BF16, isOutput=False)
    bw2 = dp("bw2", [KP, KH, OBU8], BF16, isOutput=False)
    # bias shards (o-chunk padded) f32
    tb0 = dp("tb0", [NJ_TD, 128], F32, isOutput=False)
    tb1 = dp("tb1", [NJ_TD, 128], F32, isOutput=False)
    bb0 = dp("bb0", [NJ_BU, 128], F32, isOutput=False)
    bb1 = dp("bb1", [NJ_BU, 128], F32, isOutput=False)
    bb2 = dp("bb2", [NJ_BU, 128], F32, isOutput=False)
    # pre-transposed input x: [t, 98, k, B] bf16 (partition-major)
    xt_in = dp("xt", [T, KP, KX, B], BF16, isOutput=False)
    # conv weights: pair-tap packed [node, dy, 96, co] + single-tap [.., 48, co]
    wg2_in = dp("wg2", [N, 3, KPAIR, 2 * HID], BF16, isOutput=False)
    wg1_in = dp("wg1", [N, 3, CIN, 2 * HID], BF16, isOutput=False)
    wc2_in = dp("wc2", [N, 3, KPAIR, HID], BF16, isOutput=False)
    wc1_in = dp("wc1", [N, 3, CIN, HID], BF16, isOutput=False)
    bg_in = dp("bg", [N, 2 * HID], F32, isOutput=False)
    bc_in = dp("bc", [N, HID], F32, isOutput=False)
    # fc (fc1 output-sharded: this core's OFC output columns)
    fc1_in = dp("fc1t", [KP, KH, OFC], BF16, isOutput=False)
    fc1b_in = dp("fc1b", [100, 1], F32, isOutput=False)
    fc2_in = dp("fc2t", [100, 10], F32, isOutput=False)
    fc2b_in = dp("fc2b", [10, 1], F32, isOutput=False)
    ident_in = dp("ident", [32, 32], BF16, isOutput=False)
    out_ext = dp("out", [10, B], F32, isOutput=True)
    dbg_ext = dp("dbg", [N, HID, 14, 14, B], F32, isOutput=True) if debug_h else None

    from contextlib import ExitStack
    with tile.TileContext(nc) as tc, ExitStack() as ctx:
        consts = ctx.enter_context(tc.tile_pool(name="consts", bufs=1))
        wtd_pool = ctx.enter_context(tc.tile_pool(name="wtd", bufs=4))
        wbu_pool = ctx.enter_context(tc.tile_pool(name="wbu", bufs=2))
        mpt_pool = ctx.enter_context(tc.tile_pool(name="mpt", bufs=4))
        pst_pool = ctx.enter_context(tc.tile_pool(name="pst", bufs=2, space="PSUM"))
        acc_pool = ctx.enter_context(tc.tile_pool(name="accp", bufs=1, space="PSUM"))
        conv_pool = ctx.enter_context(tc.tile_pool(name="convp", bufs=2, space="PSUM"))
        sbacc_pool = ctx.enter_context(tc.tile_pool(name="sbacc", bufs=1))
        outt_pool = ctx.enter_context(tc.tile_pool(name="outt", bufs=2))
        dram = ctx.enter_context(tc.tile_pool(name="dram", bufs=1, space="DRAM"))

        # ---------------- constants ----------------
        ident = consts.tile([32, 32], BF16)
        nc.sync.dma_start(ident[:], ident_in[:])
        wg2_sb = consts.tile([KPAIR, N, 3, 2 * HID], BF16)
        nc.sync.dma_start(wg2_sb[:], wg2_in[:].rearrange("n s c o -> c n s o"))
        wg1_sb = consts.tile([CIN, N, 3, 2 * HID], BF16)
        nc.sync.dma_start(wg1_sb[:], wg1_in[:].rearrange("n s c o -> c n s o"))
        wc2_sb = consts.tile([KPAIR, N, 3, HID], BF16)
        nc.sync.dma_start(wc2_sb[:], wc2_in[:].rearrange("n s c o -> c n s o"))
        wc1_sb = consts.tile([CIN, N, 3, HID], BF16)
        nc.sync.dma_start(wc1_sb[:], wc1_in[:].rearrange("n s c o -> c n s o"))
        bg_sb = consts.tile([2 * HID, N], F32)
        nc.sync.dma_start(bg_sb[:], bg_in[:].rearrange("n o -> o n"))
        bc_sb = consts.tile([HID, N], F32)
        nc.sync.dma_start(bc_sb[:], bc_in[:].rearrange("n o -> o n"))
        tb0_sb = consts.tile([128, NJ_TD], F32)
        nc.sync.dma_start(tb0_sb[:], tb0[:].rearrange("j p -> p j"))
        tb1_sb = consts.tile([128, NJ_TD], F32)
        nc.sync.dma_start(tb1_sb[:], tb1[:].rearrange("j p -> p j"))
        bb0_sb = consts.tile([128, NJ_BU], F32)
        nc.sync.dma_start(bb0_sb[:], bb0[:].rearrange("j p -> p j"))
        bb1_sb = consts.tile([128, NJ_BU], F32)
        nc.sync.dma_start(bb1_sb[:], bb1[:].rearrange("j p -> p j"))
        bb2_sb = consts.tile([128, NJ_BU], F32)
        nc.sync.dma_start(bb2_sb[:], bb2[:].rearrange("j p -> p j"))
        fc2_sb = consts.tile([100, 10], F32)
        nc.sync.dma_start(fc2_sb[:], fc2_in[:])
        fc1b_sb = consts.tile([100, 1], F32)
        nc.sync.dma_start(fc1b_sb[:], fc1b_in[:])
        fc2b_sb = consts.tile([10, 1], F32)
        nc.sync.dma_start(fc2b_sb[:], fc2b_in[:])
        xt_all = consts.tile([KP, T, KX, B], BF16)
        nc.sync.dma_start(xt_all[:], xt_in[:].rearrange("t p k b -> p t k b"))

        # ------------- dedicated activation tensors (shared/aliased) -------
        h = [consts.tile([HID, YP, XP, B], BF16, name=f"h{i}", tag=f"h{i}")
             for i in range(N)]
        # conv input, pair-tap stacked: [0:48]=comb, [64:112]=comb shifted
        # +1 x; [48:64] stays zero (partition bases must be 0/32/64/96)
        comb = consts.tile([KPAIR, YP, XP, B], BF16)
        rz = consts.tile([2 * HID, YP, XP, B], BF16)  # gates; [0:HID] doubles
        #   as cand / maxpool output / relu buffer
        bu_buf = consts.tile([IND, YP, XP, B], GDT)   # shared bu reload buffer
        td_buf = [consts.tile([CIN, YP, XP, B], GDT, name=f"td{i}", tag=f"td{i}")
                  for i in range(2)]
        for tt in h + td_buf + [rz, bu_buf, comb]:
            nc.vector.memset(tt[:], 0.0)

        # maxpool scratch (dedicated, so mpT never conflicts with cells);
        # tmq2 isolates h2's x-pass, which crosses the round boundary
        tmq = consts.tile([HID, YP, XP, B], BF16, name="tmq", tag="tmq")
        tmq2 = consts.tile([HID, YP, XP, B], BF16, name="tmq2", tag="tmq2")
        mp = consts.tile([HID, YP, XP, B], BF16, name="mpq", tag="mpq")

        # ---------------- helpers ----------------
        def mp_x_pass(src, tq=None, y0=1, y1=15, eng=None):
            tq = tmq if tq is None else tq
            eng = nc.vector if eng is None else eng
            eng.tensor_max(tq[0:HID, y0:y1, 2:14, :], src[0:HID, y0:y1, 1:13, :], src[0:HID, y0:y1, 2:14, :])
            eng.tensor_max(tq[0:HID, y0:y1, 2:14, :], tq[0:HID, y0:y1, 2:14, :], src[0:HID, y0:y1, 3:15, :])
            eng.tensor_max(tq[0:HID, y0:y1, 1:2, :], src[0:HID, y0:y1, 1:2, :], src[0:HID, y0:y1, 2:3, :])
            eng.tensor_max(tq[0:HID, y0:y1, 14:15, :], src[0:HID, y0:y1, 13:14, :], src[0:HID, y0:y1, 14:15, :])

        def mp_y_pass(s, tq=None):
            tq = tmq if tq is None else tq
            if s == 0:
                nc.vector.tensor_max(mp[0:HID, 2:8, 1:15, :], tq[0:HID, 1:7, 1:15, :], tq[0:HID, 2:8, 1:15, :])
                nc.vector.tensor_max(mp[0:HID, 2:8, 1:15, :], mp[0:HID, 2:8, 1:15, :], tq[0:HID, 3:9, 1:15, :])
                nc.vector.tensor_max(mp[0:HID, 1:2, 1:15, :], tq[0:HID, 1:2, 1:15, :], tq[0:HID, 2:3, 1:15, :])
            else:
                nc.vector.tensor_max(mp[0:HID, 8:14, 1:15, :], tq[0:HID, 7:13, 1:15, :], tq[0:HID, 8:14, 1:15, :])
                nc.vector.tensor_max(mp[0:HID, 8:14, 1:15, :], mp[0:HID, 8:14, 1:15, :], tq[0:HID, 9:15, 1:15, :])
                nc.vector.tensor_max(mp[0:HID, 14:15, 1:15, :], tq[0:HID, 13:14, 1:15, :], tq[0:HID, 14:15, 1:15, :])

        def transpose_slab(src, s, mt):
            """slab s of src[0:HID] (y rows 1+7s .. 8+7s) -> mt [98, HID, B].
            PE transpose needs a single-free-dim input, so first repack the
            (y, x)-strided valid slice contiguously per batch."""
            y0 = 1 + 7 * s
            stg = mpt_pool.tile([HID, B, KP], BF16, tag="stg", name="stg", bufs=2)
            nc.vector.tensor_copy(
                stg[:].rearrange("c b (y x) -> c y x b", y=7, x=14),
                src[0:HID, y0:y0 + 7, 1:15, :])
            for b in range(B):
                pt = pst_pool.tile([128, HID], BF16, tag="psT", name="ptt")
                nc.tensor.transpose(pt[:KP, 0:HID], stg[:, b, :].opt(),
                                    ident[0:HID, 0:HID])
                nc.scalar.activation(mt[:, 0:HID, b], pt[:KP, 0:HID], AF.Copy)

        def maxpool_transpose(src):
            """maxpool3x3(SAME) of src[0:HID] -> ([mt0, mt1], finish_slab1).
            Emits slab 0 immediately; the returned callback emits slab 1 and
            is passed to big_matmul's mid= hook so the first half of the
            (s-major) contraction overlaps slab 1's production."""
            out = [mpt_pool.tile([KP, HID, B], BF16, tag="mpt", name=f"mpt{s}")
                   for s in range(2)]
            tq = x_done.pop(id(src), None)
            if tq is None:
                tq = tmq
                mp_x_pass(src, tq)
            mp_y_pass(0, tq)
            transpose_slab(mp, 0, out[0])

            def finish():
                mp_y_pass(1, tq)
                transpose_slab(mp, 1, out[1])
            return out, finish

        def transpose_feat(src):
            out = [mpt_pool.tile([KP, HID, B], BF16, tag="mpt", name=f"mpt{s}")
                   for s in range(2)]
            for s in range(2):
                transpose_slab(src, s, out[s])
            return out

        def big_matmul_gen(nk, o8, nj, lhsT_of, w_dram, grp, bias_sb, agin,
                           row_off, mid=None):
            """Streamed o-sharded matmul: out.T[o8, B] = W_shard @ act (+bias),
            written o-major (bf16) into agin[row_off : row_off+o8, :].
            k order is s-major (chunks 0..nk/2-1 use activation slab 0)."""
            nslice = (o8 + 391) // 392
            pacc = acc_pool.tile([B, 512 * nslice], F32,
                                 tag=("acc" if nslice > 1 else "accbu"),
                                 name="pacc")
            for g in range(0, nk, grp):
                if mid is not None and g == nk // 2:
                    mid()
                pool = wtd_pool if o8 == OTD8 else wbu_pool
                wt = pool.tile([KP, grp, o8], BF16, tag="w", name="wt")
                nc.sync.dma_start(wt[:], w_dram[:, g:g + grp, :])
                for j in range(grp):
                    k = g + j
                    for sl in range(nslice):
                        o0 = sl * 392
                        ln = min(392, o8 - o0)
                        nc.tensor.matmul(
                            pacc[:, sl * 512: sl * 512 + ln],
                            lhsT_of(k).opt(),
                            wt[:, j, o0:o0 + ln].opt(),
                            start=(k == 0), stop=(k == nk - 1),
                        )
                yield
            sba = sbacc_pool.tile([B, o8], BF16, tag="sba", name="sba", bufs=2)
            if nslice > 1:
                pv = pacc[:].rearrange("b (s o) -> b s o", s=nslice)[:, :, 0:392]
                sv = sba[:].rearrange("b (s o) -> b s o", s=nslice)
                nc.scalar.activation(sv, pv, AF.Copy)
            else:
                nc.scalar.activation(sba[:], pacc[:, 0:o8], AF.Copy)
            outT = outt_pool.tile([128, nj, B], GDT, tag="outT", name="outT")
            for jj in range(nj):
                w_ = min(128, o8 - jj * 128)
                pt = pst_pool.tile([128, HID], BF16, tag="psT", name="pt2")
                nc.tensor.transpose(pt[:w_, 0:B], sba[:, jj * 128: jj * 128 + w_],
                                    ident[0:B, 0:B])
                nc.scalar.activation(outT[:w_, jj, :], pt[:w_, 0:B], AF.Identity,
                                     bias=bias_sb[0:w_, jj:jj + 1])
            jf = o8 // 128
            rem = o8 - jf * 128
            if jf:
                nc.gpsimd.dma_start(
                    agin[row_off: row_off + jf * 128, :].rearrange(
                        "(j p) b -> p j b", j=jf),
                    outT[:, 0:jf, :])
            if rem:
                nc.gpsimd.dma_start(
                    agin[row_off + jf * 128: row_off + o8, :],
                    outT[0:rem, jf, :])

        def big_matmul(*a, **kw):
            for _ in big_matmul_gen(*a, **kw):
                pass

        def do_gather(agin, agout):
            if no_cc:
                for c in range(NCORES):
                    nc.gpsimd.dma_start(agout[c], agin[:])
            else:
                nc.gpsimd.collective_compute(
                    "AllGather", mybir.AluOpType.bypass,
                    replica_groups=[list(range(NCORES))],
                    ins=[agin.opt()], outs=[agout.opt()])

        def reload(buf, agout, row_off, nch_l):
            """agout [8, rows, B] o-major (bf16) -> buf [8*nch_l, 16, 16, B]."""
            for c in range(NCORES):
                srcv = agout[c, row_off: row_off + nch_l * 196, :].rearrange(
                    "(chl y x) b -> chl y x b", chl=nch_l, y=14, x=14)
                nc.scalar.dma_start(
                    buf[nch_l * c: nch_l * (c + 1), 1:15, 1:15, :], srcv)

        def shift_half(y0, y1, c0=0, c1=CIN):
            """comb[64+c0:64+c1, f] = comb[c0:c1, f+B] for y-rows [y0, y1).
            c0/c1 in {0, 32, 48} keep partition bases on the 0/32/64/96
            grid."""
            cf = comb[:].rearrange("c y x b -> c (y x b)")
            f0 = max(0, y0 * XP * B - B)
            f1 = min(FLAT - B, y1 * XP * B - B)
            nc.scalar.activation(cf[64 + c0:64 + c1, f0:f1],
                                 cf[c0:c1, f0 + B:f1 + B], AF.Copy)

        def conv6_gen(w2_of, w1_of, nco, bias_ap, out_t, act_fn):
            inp_f = comb[:].rearrange("c y x b -> c (y x b)")
            out_f = out_t.rearrange("c y x b -> c (y x b)")
            q = 0
            while q < WLEN:
                ln = min(512, WLEN - q)
                pc = conv_pool.tile([nco, 512], F32, tag="conv", name="pc")
                for i, dy in enumerate((-1, 0, 1)):
                    offp = (dy * XP - 1) * B     # pair taps (dy,-1)+(dy,0)
                    nc.tensor.matmul(
                        pc[:, 0:ln],
                        w2_of(i).opt(),
                        inp_f[0:KPAIR, WSTART + q + offp: WSTART + q + offp + ln],
                        start=(i == 0), stop=False,
                    )
                    offs = (dy * XP + 1) * B     # single tap (dy,+1)
                    nc.tensor.matmul(
                        pc[:, 0:ln],
                        w1_of(i).opt(),
                        inp_f[0:CIN, WSTART + q + offs: WSTART + q + offs + ln],
                        start=False, stop=(i == 2),
                    )
                nc.scalar.activation(out_f[:, WSTART + q: WSTART + q + ln],
                                     pc[:, 0:ln], act_fn, bias=bias_ap)
                q += ln
                yield

        x_done = {}   # id(h tile) -> scratch holding its x-pass

        def cell_gen(node, td_t, xq=None):
            """GRU cell update of h[node] from bu_buf (+ td_t), as a generator
            so independent matmul streams can interleave. All elementwise work
            is split into y-halves to shorten the serial chain into each conv.
            If xq is given, the maxpool x-pass of the fresh h is emitted per
            y-half right behind the update, for the round's next mpT."""
            hh = h[node]
            for y0, y1 in ((0, 9), (9, 16)):
                nc.vector.tensor_copy(comb[0:HID, y0:y1, :, :], hh[:, y0:y1, :, :])
                nc.vector.tensor_copy(comb[HID:CIN, y0:y1, :, :],
                                      bu_buf[:, y0:y1, :, :])
                if td_t is not None:
                    nc.vector.tensor_add(comb[0:CIN, y0:y1, :, :],
                                         comb[0:CIN, y0:y1, :, :],
                                         td_t[:, y0:y1, :, :])
                shift_half(y0, y1)
                yield
            yield from conv6_gen(lambda i: wg2_sb[:, node, i, :],
                                 lambda i: wg1_sb[:, node, i, :],
                                 2 * HID, bg_sb[:, node:node + 1], rz[:],
                                 AF.Sigmoid)
            # comb -> cand-conv input: [r*h, bu]
            for y0, y1 in ((0, 9), (9, 16)):
                nc.vector.tensor_mul(comb[0:HID, y0:y1, :, :],
                                     rz[0:HID, y0:y1, :, :], hh[:, y0:y1, :, :])
                if td_t is not None:
                    nc.vector.tensor_copy(comb[HID:CIN, y0:y1, :, :],
                                          bu_buf[:, y0:y1, :, :])
                shift_half(y0, y1)
                yield
            # cand -> rz[0:HID] (r no longer needed)
            yield from conv6_gen(lambda i: wc2_sb[:, node, i, :],
                                 lambda i: wc1_sb[:, node, i, :],
                                 HID, bc_sb[:, node:node + 1],
                                 rz[0:HID, :, :, :], AF.Tanh)
            for y0, y1 in ((1, 9), (9, 15)):
                hv = hh[:, y0:y1, 1:15, :]
                cv = rz[0:HID, y0:y1, 1:15, :]
                # z lives at base partition 32; DVE tensor-tensor ops need
                # equal base partitions, so stage it at base 0 in comb.
                zc = comb[0:HID, y0:y1, 1:15, :]
                nc.vector.tensor_copy(zc, rz[HID:2 * HID, y0:y1, 1:15, :])
                nc.vector.tensor_sub(cv, cv, hv)
                nc.vector.tensor_mul(cv, cv, zc)
                nc.vector.tensor_add(hv, hv, cv)
                if xq is not None:
                    mp_x_pass(hh, xq, y0, y1)
                yield
            if xq is not None:
                x_done[id(hh)] = xq

        def cell(node, td_t, xq=None):
            for _ in cell_gen(node, td_t, xq):
                pass

        def zip2(ga, gb, ratio=2):
            """Interleave two generators, giving `ga` (the critical-path
            cell) `ratio` steps per `gb` step."""
            while ga is not None or gb is not None:
                for _ in range(ratio):
                    if ga is not None:
                        try:
                            next(ga)
                        except StopIteration:
                            ga = None
                if gb is not None:
                    try:
                        next(gb)
                    except StopIteration:
                        gb = None

        # ------------- round schedule: 2 collectives per timestep -------------
        # Round u (u = timestep of cell1/cell2):
        #   B(u): gather {td1(u) [u>=2], bu1(u)} -> cell1(u)
        #   A(u): gather {bu2(u) [u>=1], bu0(u+1) [u+1<T], td0(u+1) [2<=u+1<T]}
        #         -> cell2(u) [u>=1], cell0(u+1) [u+1<T]
        # bu2(u) and td0(u+1) share mp(h1@u) (one maxpool+transpose).
        # cell0(u+1) is returned as a pending generator and interleaved with
        # td1(u+1)'s matmul stream at the start of the next round_B.
        def lam(m):
            return lambda k, mm=m: mm[k // HID][:, (k % HID), :]

        def round_A_pre(u):
            hbu2 = 1 <= u < t_end
            hbu0 = u + 1 < min(T, t_end)
            htd0 = 2 <= u + 1 < min(T, t_end)
            rows = ((OBU8 if hbu2 else 0) + (OBU8 if hbu0 else 0)
                    + (OTD8 if htd0 else 0))
            if rows == 0:
                return None
            agin = dram.tile([rows, B], GDT, name=f"aginA_{u}", tag=f"aginA_{u}")
            st = dict(hbu2=hbu2, hbu0=hbu0, htd0=htd0, rows=rows, agin=agin,
                      ro_bu2=0, ro_bu0=(OBU8 if hbu2 else 0))
            st["ro_td0"] = st["ro_bu0"] + (OBU8 if hbu0 else 0)
            if hbu0:
                # bu0 depends only on x: its matmuls slot into the PE-idle
                # window while round_B's gather + reload are in flight.
                big_matmul(KX, OBU8, NJ_BU,
                           lambda k: xt_all[:, u + 1, k, :], bw0, KX,
                           bb0_sb, agin, st["ro_bu0"])
            return st

        def round_A_rest(u, st):
            if st is None:
                return None
            agin = st["agin"]
            hbu2, hbu0, htd0 = st["hbu2"], st["hbu0"], st["htd0"]
            mid1 = None
            if hbu2 or htd0:
                m1, mid1 = maxpool_transpose(h[1])
            if hbu2:
                big_matmul(KH, OBU8, NJ_BU, lam(m1), bw2, GRP_BU,
                           bb2_sb, agin, st["ro_bu2"], mid=mid1)
                mid1 = None
            if htd0:
                big_matmul(KH, OTD8, NJ_TD, lam(m1), tw0, GRP_TD,
                           tb0_sb, agin, st["ro_td0"], mid=mid1)
                mid1 = None
            if mid1 is not None:
                mid1()
            agout = dram.tile([NCORES, st["rows"], B], GDT, name=f"agoutA_{u}",
                              tag=f"agoutA_{u}",
                              addr_space="Local" if no_cc else "Shared")
            do_gather(agin, agout)
            if hbu2:
                reload(bu_buf, agout, st["ro_bu2"], IND // NCORES)
                cell(2, None, xq=tmq2)
            if htd0:
                reload(td_buf[0], agout, st["ro_td0"], CIN // NCORES)
            if hbu0:
                reload(bu_buf, agout, st["ro_bu0"], IND // NCORES)
                return cell_gen(0, td_buf[0] if htd0 else None, xq=tmq)
            return None

        def round_B(u, pend_cell0):
            htd1 = u >= 2
            rows = (OTD8 if htd1 else 0) + OBU8
            agin = dram.tile([rows, B], GDT, name=f"aginB_{u}", tag=f"aginB_{u}")
            ro = 0
            if htd1:
                m2, mid2 = maxpool_transpose(h[2])
                td1_gen = big_matmul_gen(KH, OTD8, NJ_TD, lam(m2), tw1, GRP_TD,
                                         tb1_sb, agin, ro, mid=mid2)
                ro += OTD8
            else:
                td1_gen = None
            zip2(pend_cell0, td1_gen)
            m0, mid0 = maxpool_transpose(h[0])
            big_matmul(KH, OBU8, NJ_BU, lam(m0), bw1, GRP_BU,
                       bb1_sb, agin, ro, mid=mid0)
            agout = dram.tile([NCORES, rows, B], GDT, name=f"agoutB_{u}",
                              tag=f"agoutB_{u}",
                              addr_space="Local" if no_cc else "Shared")
            do_gather(agin, agout)
            stA = round_A_pre(u)   # bu0(u+1) fills the gather window
            if htd1:
                reload(td_buf[1], agout, 0, CIN // NCORES)
            reload(bu_buf, agout, ro, IND // NCORES)
            cell(1, td_buf[1] if htd1 else None, xq=tmq)
            return stA

        pend = round_A_rest(-1, round_A_pre(-1))  # bootstrap: bu0(0)->cell0(0)
        for u in range(t_end):
            if u >= 1:
                stA = round_B(u, pend)
                pend = None
            else:
                zip2(pend, None)
                pend = None
                stA = round_A_pre(u)
            pend = round_A_rest(u, stA)
        zip2(pend, None)

        if debug_h:
            for i in range(N):
                nc.gpsimd.dma_start(dbg_ext[i], h[i][:, 1:15, 1:15, :])
        # -------- final FC head (fc1 output-sharded + partial gather) --------
        # Each core computes its OFC of the 104 (padded) fc1 outputs with the
        # full contraction, then one tiny AllGather assembles p1.
        nc.scalar.activation(rz[0:HID, :, :, :], h[2][:], AF.Relu)
        pT = transpose_feat(rz)
        pfc = acc_pool.tile([OFC, 16], F32, tag="acc", name="pfc")
        for g in range(0, KH, 8):
            wf = wtd_pool.tile([KP, 8, OFC], BF16, tag="w", name="wf")
            nc.sync.dma_start(wf[:], fc1_in[:, g:g + 8, :])
            for j in range(8):
                k = g + j
                nc.tensor.matmul(pfc[:], wf[:, j, :].opt(),
                                 pT[k // HID][:, (k % HID), :].opt(),
                                 start=(k == 0), stop=(k == KH - 1))
        pfs = sbacc_pool.tile([OFC, 16], F32, tag="pfs", name="pfs")
        nc.scalar.activation(pfs[:], pfc[:], AF.Copy)
        aginF = dram.tile([OFC, 16], F32, name="aginF", tag="aginF")
        nc.gpsimd.dma_start(aginF[:], pfs[:])
        agoutF = dram.tile([NCORES, OFC, 16], F32, name="agoutF", tag="agoutF",
                           addr_space="Local" if no_cc else "Shared")
        do_gather(aginF, agoutF)
        p1r = sbacc_pool.tile([NCORES * OFC, 16], F32, tag="p1r", name="p1r")
        nc.gpsimd.dma_start(p1r[:], agoutF[:].rearrange("c o b -> (c o) b"))
        p1 = sbacc_pool.tile([100, 16], F32, tag="p1", name="p1")
        nc.scalar.activation(p1[:], p1r[0:100, :], AF.Relu,
                             bias=fc1b_sb[:])
        pf2 = acc_pool.tile([128, HID], F32, tag="acc", name="pf2")
        nc.tensor.matmul(pf2[0:10, 0:16], fc2_sb[:], p1[:],
                         start=True, stop=True)
        osb = sbacc_pool.tile([10, 16], F32, tag="osb", name="osb")
        nc.scalar.activation(osb[:], pf2[0:10, 0:16], AF.Identity,
                             bias=fc2b_sb[:])
        nc.gpsimd.dma_start(out_ext[:], osb[:])

    nc.finalize()
    return nc


# ---------------------------------------------------------------- host ----
def _feat_perm(nch):
    """Device feature order (ch, s, p) -> torch flat feature index."""
    perm = np.zeros((nch * 2, KP), np.int64)
    for ch in range(nch):
        for s in range(2):
            k = s * nch + ch
            p = np.arange(KP)
            y = s * 7 + p // 14
            x = p % 14
            perm[k] = ch * 196 + y * 14 + x
    return perm


def _shard_w(wmat, nch_in, o8):
    """wmat (O, K) torch-order -> per-core [98, nk, o8] bf16 shards."""
    perm = _feat_perm(nch_in)
    wt = wmat.T[perm.reshape(-1)].reshape(perm.shape[0], KP,
                                          wmat.shape[0]).astype(NPBF)
    return [np.ascontiguousarray(wt[:, :, c * o8:(c + 1) * o8].transpose(1, 0, 2))
            for c in range(NCORES)]


def _pad_bias(bvec, o8, nj):
    out = []
    for c in range(NCORES):
        bp = np.zeros((nj, 128), np.float32)
        bp.reshape(-1)[:o8] = bvec[c * o8:(c + 1) * o8]
        out.append(bp)
    return out


def prep_inputs(inputs):
    x = np.asarray(inputs["x"], np.float32)
    permx = _feat_perm(C)
    xt = np.zeros((T, KP, KX, B), NPBF)
    for t in range(T):
        flat = x[:, t].reshape(B, C * 196).T      # [588, B]
        xt[t] = flat[permx.reshape(-1)].reshape(KX, KP, B).transpose(1, 0, 2)

    # td outputs are reloaded straight into device channel order [h, bu]:
    # permute td_w / td_b output rows from torch order [bu, h] accordingly.
    ci_out = np.concatenate([np.arange(IND, CIN), np.arange(0, IND)])
    o_perm = (ci_out[:, None] * 196 + np.arange(196)[None, :]).reshape(-1)
    tw0 = _shard_w(np.asarray(inputs["td_w0"], np.float32)[o_perm], HID, OTD8)
    tw1 = _shard_w(np.asarray(inputs["td_w1"], np.float32)[o_perm], HID, OTD8)
    bw0 = _shard_w(np.asarray(inputs["bu_w0"], np.float32), C, OBU8)
    bw1 = _shard_w(np.asarray(inputs["bu_w1"], np.float32), HID, OBU8)
    bw2 = _shard_w(np.asarray(inputs["bu_w2"], np.float32), HID, OBU8)
    tb0 = _pad_bias(np.asarray(inputs["td_b0"], np.float32)[o_perm], OTD8, NJ_TD)
    tb1 = _pad_bias(np.asarray(inputs["td_b1"], np.float32)[o_perm], OTD8, NJ_TD)
    bb0 = _pad_bias(np.asarray(inputs["bu_b0"], np.float32), OBU8, NJ_BU)
    bb1 = _pad_bias(np.asarray(inputs["bu_b1"], np.float32), OBU8, NJ_BU)
    bb2 = _pad_bias(np.asarray(inputs["bu_b2"], np.float32), OBU8, NJ_BU)

    # conv weights: device ci order = [h (0:32) -> torch ci 16..47, bu -> 0..15]
    # packed for 2-tap passes: wg2[n, iy] rows 0:48 = tap (dy,-1),
    # rows 48:96 = tap (dy,0); wg1[n, iy] = tap (dy,+1).
    ci_perm = np.concatenate([np.arange(IND, CIN), np.arange(0, IND)])
    Wg = np.asarray(inputs["Wg"], np.float32)
    Wc = np.asarray(inputs["Wc"], np.float32)
    wg2 = np.zeros((N, 3, KPAIR, 2 * HID), NPBF)
    wg1 = np.zeros((N, 3, CIN, 2 * HID), NPBF)
    wc2 = np.zeros((N, 3, KPAIR, HID), NPBF)
    wc1 = np.zeros((N, 3, CIN, HID), NPBF)
    for iy in range(3):
        for n in range(N):
            wg2[n, iy, 0:CIN] = Wg[n][:, ci_perm, iy, 0].T
            wg2[n, iy, 64:] = Wg[n][:, ci_perm, iy, 1].T
            wg1[n, iy] = Wg[n][:, ci_perm, iy, 2].T
            wc2[n, iy, 0:CIN] = Wc[n][:, ci_perm, iy, 0].T
            wc2[n, iy, 64:] = Wc[n][:, ci_perm, iy, 1].T
            wc1[n, iy] = Wc[n][:, ci_perm, iy, 2].T

    permh = _feat_perm(HID)
    fc1 = np.asarray(inputs["fc1_w"], np.float32)     # (100, 6272)
    fc1p = np.zeros((NCORES * OFC, fc1.shape[1]), np.float32)
    fc1p[0:100] = fc1
    # [6272, 104] -> perm -> [KH, KP, 104] -> [KP, KH, 104] bf16, col-sharded
    fc1t_full = fc1p.T[permh.reshape(-1)].reshape(KH, KP, NCORES * OFC)
    fc1t_full = fc1t_full.transpose(1, 0, 2).astype(NPBF)
    fc1t = [np.ascontiguousarray(fc1t_full[:, :, c * OFC:(c + 1) * OFC])
            for c in range(NCORES)]
    fc2t = np.ascontiguousarray(np.asarray(inputs["fc2_w"], np.float32).T)  # (100, 10)

    common = {
        "xt": xt,
        "wg2": wg2, "wg1": wg1, "wc2": wc2, "wc1": wc1,
        "bg": np.asarray(inputs["bg"], np.float32),
        "bc": np.asarray(inputs["bc"], np.float32),
        "fc1b": np.asarray(inputs["fc1_b"], np.float32).reshape(100, 1),
        "fc2t": fc2t,
        "fc2b": np.asarray(inputs["fc2_b"], np.float32).reshape(10, 1),
        "ident": np.eye(32, dtype=np.float32).astype(NPBF),
    }
    in_maps = []
    for c in range(NCORES):
        m = dict(common)
        m.update({
            "tw0": tw0[c], "tw1": tw1[c], "bw0": bw0[c],
            "bw1": bw1[c], "bw2": bw2[c],
            "tb0": tb0[c], "tb1": tb1[c],
            "bb0": bb0[c], "bb1": bb1[c],
            "bb2": bb2[c], "fc1t": fc1t[c],
        })
        in_maps.append(m)
    return in_maps


def get_graph():
    if "nc" not in _CACHED:
        _CACHED["nc"] = build_graph()
    return _CACHED["nc"]


def kernel(**inputs):
    nc = get_graph()
    in_maps = prep_inputs(inputs)
    res = bass_utils.run_bass_kernel_spmd(nc, in_maps, core_ids=list(range(NCORES)))
    out_t = np.asarray(res.results[0]["out"]).reshape(10, B)
    return np.ascontiguousarray(out_t.T).astype(np.float32)

